# revision 49
# baseline (speedup 1.0000x reference)
"""ColorQuantizer (VQ nearest-palette-color) Trainium2 Bass kernel, v8.

out[b,:,h,w] = palette[argmin_k ||(x+0.01*noise)[b,:,h,w] - palette[k]||]

Score trick: argmin_k ||y - p_k||^2 == argmin_k (d_k - 2 p_k . y) with
d_k = ||p_k||^2 -- an affine function of y:
    t_k = y0 * a0_k + d_k          [ScalarE ACT Identity: in*scale+bias]
    t_k += y1 * a1_k               [STT on DVE, or ACT mult + Pool add]
    t_k += y2 * a2_k
with a_ck = -2*palette[k,c].

Selection trick: palette colors are quantized to 7 bits/channel and packed
into one fp32-exact integer (q0*65536 + q1*256 + q2, all < 2^23), so the
running-argmin selection is a single copy_predicated per color; the packed
winner is decoded once per chunk with exact rne field extractions (the gap
bit per field keeps every fraction < 0.5). Max color error 0.5/127
(~4e-3) against a 2e-2 rel-err budget.

Engine facts (BIR verifier + measured):
  - scalar_tensor_tensor (TensorScalarPtr) is DVE-only; Pool rejects it
  - Pool tensor_tensor requires matching dtypes on all operands
  - CopyPredicated needs an integer mask -> is_lt (fp32 -> uint8) is DVE-only
  - Pool 2-input tensor_tensor is ~2x slower than DVE; 1-input tensor_scalar
    is roughly line-rate
So: DVE carries STT/is_lt/copy_predicated; Pool takes min + integer decode
extractions + broadcast copies; ScalarE takes the first score MAC and some
single-input multiplies. A cost-aware greedy balancer decides, per MAC,
between a fused DVE STT and a decomposed ACT-multiply + Pool-add.

Scheduling: chunks of [128, 1024] are processed in interleaved pairs (two
independent selection chains alternate inside each in-order engine queue),
and each pair's decode is emitted one pair late so its selection-gated ops
never block a queue head while the next pair's work is ready.

Sharding: pure data parallel over batch (32 -> 8 cores x 4), palette
replicated. Repeat (benchmark) is a hardware For_i loop so program size is
independent of repeat count.
"""
import sys

sys.path.insert(0, "/opt/trn_rl_repo")

import numpy as np

import concourse.bacc as bacc
import concourse.mybir as mybir
from concourse.tile import TileContext
from concourse.bass_utils import run_bass_kernel_spmd

B, C, H, W = 32, 3, 512, 512
K = 16
N_CORES = 8
B_PER_CORE = B // N_CORES  # 4
NOISE_SCALE = 0.01

F = 1024            # free-dim elements per [128, F] chunk (half a channel plane)
CHUNKS = B_PER_CORE * 2048 // F   # 8 chunks per core, processed in pairs
QBITS = 7
QMAX = (1 << QBITS) - 1  # 127

_DT = mybir.dt.float32
_IT = mybir.dt.int32
Alu = mybir.AluOpType
Act = mybir.ActivationFunctionType

# measured per-op costs (ns, [128,1024] operands) used by the greedy balancer
# (device microbench, repeat-slope)
COSTS = {
    "dve": 1250.0,      # 2-input DVE op (TT/STT/copy_predicated)
    "dve_ts": 800.0,    # 1-input DVE tensor_scalar / tensor_copy (2x mode)
    "pool_tt": 2500.0,  # Pool 2-input tensor_tensor (only add/sub/mult work)
    "act": 1170.0,      # ScalarE activation
}


class _Balance:
    """Cost-aware greedy splitter across DVE / Pool / ScalarE."""

    def __init__(self, nc):
        self.nc = nc
        self.load = {"dve": 0.0, "pool": 0.0, "act": 0.0}

    def _charge(self, eng, key):
        self.load[eng] += COSTS[key]

    def mac(self, out, acc, yin, scale_ap, t_scratch):
        """out = acc + yin * scale. Fused DVE STT, or ACT mult + Pool add."""
        dve_done = self.load["dve"] + COSTS["dve"]
        dec_done = max(self.load["act"] + COSTS["act"],
                       self.load["pool"] + COSTS["pool_tt"])
        if dve_done <= dec_done or t_scratch is None:
            self._charge("dve", "dve")
            self.nc.vector.scalar_tensor_tensor(
                out=out, in0=yin, scalar=scale_ap, in1=acc,
                op0=Alu.mult, op1=Alu.add)
        else:
            self._charge("act", "act")
            self._charge("pool", "pool_tt")
            self.nc.scalar.activation(
                out=t_scratch, in_=yin, func=Act.Identity, scale=scale_ap)
            self.nc.gpsimd.tensor_tensor(
                out=out, in0=acc, in1=t_scratch, op=Alu.add)

    def tt_min(self, out, in0, in1):
        # Pool's Q7 kernels only implement add/sub/mult -> min is DVE-only
        self._charge("dve", "dve")
        self.nc.vector.tensor_tensor(out=out, in0=in0, in1=in1, op=Alu.min)

    def dve(self):
        self._charge("dve", "dve")
        return self.nc.vector

    def dve_ts(self):
        self._charge("dve", "dve_ts")
        return self.nc.vector


def _chunk3(t_dram, j):
    """[128, 3, F] DRAM view: chunk j (of CHUNKS) across all 3 channels."""
    b, h = divmod(j, 2048 // F)
    v = t_dram[b].rearrange("c (p a) w -> p c (a w)", p=128)
    return v[:, :, h * F : (h + 1) * F]


def _t3(tile):
    """[128, 3, F] view of a [128, 3*F] 3-channel tile."""
    return tile[:].rearrange("p (c f) -> p c f", c=C)


def _build(repeat=1, num_devices=N_CORES):
    nc = bacc.Bacc("TRN2", target_bir_lowering=False, debug=False,
                   num_devices=num_devices)
    x = nc.dram_tensor("x", [B_PER_CORE, C, H, W], _DT, kind="ExternalInput").ap()
    n = nc.dram_tensor("noise", [B_PER_CORE, C, H, W], _DT, kind="ExternalInput").ap()
    pal = nc.dram_tensor("palette", [K, C], _DT, kind="ExternalInput").ap()
    o = nc.dram_tensor("out", [B_PER_CORE, C, H, W], _DT, kind="ExternalOutput").ap()

    with TileContext(nc) as tc:
        with (
            tc.tile_pool(name="const", bufs=1) as cpool,
            tc.tile_pool(name="io", bufs=2) as io,
            tc.tile_pool(name="sc", bufs=2) as sc,
            tc.tile_pool(name="scr", bufs=1) as scr,
            tc.tile_pool(name="carry", bufs=2) as carry,
        ):
            # palette -> SBUF [128, 48] broadcast across partitions; col = k*3+c
            pal_sb = cpool.tile([128, K * C], _DT)
            nc.sync.dma_start(
                out=pal_sb[:],
                in_=pal.rearrange("(o k) c -> o (k c)", o=1).to_broadcast([128, K * C]),
            )
            # a = -2 * palette  (per-color per-channel scale columns)
            neg2_sb = cpool.tile([128, K * C], _DT)
            nc.vector.tensor_scalar(
                out=neg2_sb[:], in0=pal_sb[:], scalar1=-2.0, scalar2=None,
                op0=Alu.mult)
            # d_k = sum_c palette[k,c]^2  -> [128, K] columns
            sq_sb = cpool.tile([128, K * C], _DT)
            nc.vector.tensor_tensor(
                out=sq_sb[:], in0=pal_sb[:], in1=pal_sb[:], op=Alu.mult)
            d_sb = cpool.tile([128, K], _DT)
            nc.vector.tensor_reduce(
                out=d_sb[:],
                in_=sq_sb[:].rearrange("p (k c) -> p k c", k=K),
                axis=mybir.AxisListType.X, op=Alu.add)
            # packed 7-bit palette: pk = q0*65536 + q1*256 + q2, q = rne(p*127).
            # 7-bit fields at 8-bit strides leave a gap bit, so pk < 2^23
            # stays exact in fp32 and rne(pk/256^s) extractions are tie-free.
            q_sb = cpool.tile([128, K * C], _IT)
            nc.vector.tensor_scalar(
                out=q_sb[:], in0=pal_sb[:], scalar1=float(QMAX), scalar2=None,
                op0=Alu.mult)
            pk_tmp = cpool.tile([128, K], _IT)
            pk_sb = cpool.tile([128, K], _IT)
            qv = q_sb[:].rearrange("p (k c) -> p k c", k=K)
            nc.vector.scalar_tensor_tensor(
                out=pk_tmp[:], in0=qv[:, :, 0], scalar=256.0,
                in1=qv[:, :, 1], op0=Alu.mult, op1=Alu.add)
            nc.vector.scalar_tensor_tensor(
                out=pk_sb[:], in0=pk_tmp[:], scalar=256.0,
                in1=qv[:, :, 2], op0=Alu.mult, op1=Alu.add)
            # fp32 copies of pk1 - pk0 and pk0 (exact: < 2^23), for the fused
            # k0/k1 otp init (otp = mask*dpk + pk0; TS scalars must be fp32)
            pkd_sb = cpool.tile([128, 1], _DT)
            nc.vector.scalar_tensor_tensor(
                out=pkd_sb[:], in0=pk_sb[:, 0:1], scalar=-1.0,
                in1=pk_sb[:, 1:2], op0=Alu.mult, op1=Alu.add)
            pk0_sb = cpool.tile([128, 1], _DT)
            nc.vector.tensor_copy(out=pk0_sb[:], in_=pk_sb[:, 0:1])

            def emit_decode(bal, st):
                # decode packed winner -> fp32 channels into the nt3 tile.
                # rne(int->int scaled) field extraction is exact: the gap
                # bit keeps every fraction < 0.5. Emitted one pair late so
                # the selection-gated ops never block an in-order queue head
                # while the next pair's work is ready. Scratch lives in the
                # pair's (dead) t buffers via bitcast.
                for s in st:
                    ot, otp = s["ot"], s["otp"]
                    w0 = s["t"][0][:].bitcast(_IT)
                    u = s["t"][1][:].bitcast(_IT)
                    w1 = s["t"][2][:].bitcast(_IT)
                    w2 = w0  # w0 is dead once u is computed; reuse its buffer
                    bal._charge("act", "act")
                    nc.scalar.activation(
                        out=w0, in_=otp[:], func=Act.Identity,
                        scale=1.0 / 65536.0)
                    bal._charge("act", "act")
                    nc.scalar.activation(
                        out=ot[0], in_=w0, func=Act.Identity,
                        scale=1.0 / QMAX)
                    bal.dve().scalar_tensor_tensor(
                        out=u, in0=w0, scalar=-65536.0,
                        in1=otp[:], op0=Alu.mult, op1=Alu.add)
                    bal._charge("act", "act")
                    nc.scalar.activation(
                        out=w1, in_=u, func=Act.Identity,
                        scale=1.0 / 256.0)
                    bal._charge("act", "act")
                    nc.scalar.activation(
                        out=ot[1], in_=w1, func=Act.Identity,
                        scale=1.0 / QMAX)
                    bal.dve().scalar_tensor_tensor(
                        out=w2, in0=w1, scalar=-256.0,
                        in1=u, op0=Alu.mult, op1=Alu.add)
                    bal._charge("act", "act")
                    nc.scalar.activation(
                        out=ot[2], in_=w2, func=Act.Identity,
                        scale=1.0 / QMAX)
                    if s["j"] >= CHUNKS - 2:
                        # final pair: store per channel so the first store
                        # overlaps the rest of the decode chain (shortens
                        # the drain tail)
                        for c in range(C):
                            nc.sync.dma_start(
                                out=_chunk3(o, s["j"])[:, c],
                                in_=_t3(s["nt3"])[:, c])
                    else:
                        nc.sync.dma_start(out=_chunk3(o, s["j"]), in_=_t3(s["nt3"]))

            with tc.For_i(0, repeat, 1):
                pending = None
                for j0 in range(0, CHUNKS, 2):
                    # process chunks (j0, j0+1) with their k-loops
                    # interleaved: two independent selection chains
                    # alternate inside each in-order engine queue, so a
                    # dependency stall in one chain is filled by the other.
                    bal = _Balance(nc)
                    P = 2  # pair
                    st = []  # per-parity state dicts
                    for par in range(P):
                        j = j0 + par
                        xt3 = io.tile([128, C * F], _DT, tag=f"x{par}", name="xt3")
                        nt3 = io.tile([128, C * F], _DT, tag=f"n{par}", name="nt3")
                        if j0 == 0:
                            # startup: load per channel so y0 (and the 16
                            # ScalarE score MACs that only need y0) can
                            # start while channels 1-2 are still in flight
                            for c in range(C):
                                nc.sync.dma_start(
                                    out=_t3(xt3)[:, c], in_=_chunk3(x, j)[:, c])
                                nc.sync.dma_start(
                                    out=_t3(nt3)[:, c], in_=_chunk3(n, j)[:, c])
                        else:
                            nc.sync.dma_start(out=_t3(xt3), in_=_chunk3(x, j))
                            nc.sync.dma_start(out=_t3(nt3), in_=_chunk3(n, j))
                        st.append(dict(
                            j=j,
                            yt=[xt3[:, c * F : (c + 1) * F] for c in range(C)],
                            ot=[nt3[:, c * F : (c + 1) * F] for c in range(C)],
                            nt3=nt3,
                            m=carry.tile([128, F], _DT, tag=f"m{par}", name="m"),
                            mask=carry.tile([128, F], mybir.dt.uint8,
                                            tag=f"k{par}", name="mask"),
                            otp=carry.tile([128, F], _IT, tag=f"o{par}", name="otp"),
                            t=[None, None, None],
                        ))

                    # decode of the PREVIOUS pair: emitted after this pair's
                    # DMA loads but before its compute, so the decode ops
                    # (whose deps are already satisfied) sit at the FRONT of
                    # each engine queue, drain immediately, and free the
                    # previous pair's buffers early for the next loads
                    if pending is not None:
                        emit_decode(bal, pending)
                        pending = None

                    # y_c = x_c + NOISE_SCALE * n_c, in place into xt3
                    # (nt3 is dead afterwards; it stages the decoded colors)
                    for par in range(P):
                        s = st[par]
                        for c in range(C):
                            ys = scr.tile([128, F], _DT, tag=f"s{par}{c % 2}",
                                          name="ys")
                            bal.mac(s["yt"][c], s["yt"][c], s["ot"][c],
                                    NOISE_SCALE, ys[:])

                    for k in range(K):
                        for par in range(P):
                            s = st[par]
                            yt, m, mask, otp = s["yt"], s["m"], s["mask"], s["otp"]
                            # t = y0*a0 + d_k on ScalarE
                            t = sc.tile([128, F], _DT, tag=f"t{par}{k % 3}",
                                        name="t")
                            s["t"][k % 3] = t
                            ts = scr.tile([128, F], _DT, tag=f"s{par}{k % 2}",
                                          name="ts")
                            bal._charge("act", "act")
                            nc.scalar.activation(
                                out=t[:], in_=yt[0], func=Act.Identity,
                                bias=d_sb[:, k : k + 1],
                                scale=neg2_sb[:, k * C : k * C + 1])
                            bal.mac(t[:], t[:], yt[1],
                                    neg2_sb[:, k * C + 1 : k * C + 2], ts[:])
                            tm = m[:] if k == 0 else t[:]
                            bal.mac(tm, t[:], yt[2],
                                    neg2_sb[:, k * C + 2 : k * C + 3], ts[:])
                            if k == 0:
                                pass  # m = t0; selection starts at k=1
                            elif k == 1:
                                # fused init: otp = mask*(pk1-pk0) + pk0
                                # (a 1-input TS replaces init-copy + first
                                # copy_predicated)
                                bal.dve().tensor_tensor(
                                    out=mask[:], in0=t[:], in1=m[:], op=Alu.is_lt)
                                bal.tt_min(m[:], m[:], t[:])
                                bal.dve_ts().tensor_scalar(
                                    out=otp[:], in0=mask[:],
                                    scalar1=pkd_sb[:, 0:1],
                                    scalar2=pk0_sb[:, 0:1],
                                    op0=Alu.mult, op1=Alu.add)
                            else:
                                # is_lt writes a uint8 mask: mixed dtypes are
                                # DVE-only, and CopyPredicated needs an
                                # integer mask
                                bal.dve().tensor_tensor(
                                    out=mask[:], in0=t[:], in1=m[:], op=Alu.is_lt)
                                if k < K - 1:  # m is dead after the last mask
                                    bal.tt_min(m[:], m[:], t[:])
                                bal.dve().copy_predicated(
                                    out=otp[:], mask=mask[:],
                                    data=pk_sb[:, k : k + 1].to_broadcast([128, F]))

                    pending = st
                emit_decode(_Balance(nc), pending)

    nc.compile()
    return nc


_NC_CACHE = {}


def _get_nc(repeat=1):
    if repeat not in _NC_CACHE:
        _NC_CACHE[repeat] = _build(repeat)
    return _NC_CACHE[repeat]


def _subsample_check(out, x, noise, palette):
    """Validate a pixel subsample against numpy argmin (abs tol covers the
    7-bit color quantization). Catches transient device/compile garbage."""
    step = 97
    y = (np.transpose(x + NOISE_SCALE * noise, (0, 2, 3, 1))
         .reshape(-1, 3)[::step])
    got = np.transpose(out, (0, 2, 3, 1)).reshape(-1, 3)[::step]
    d = ((y[:, None, :] - palette[None, :, :]) ** 2).sum(-1)
    exp = palette[np.argmin(d, axis=-1)]
    bad = (np.abs(got - exp) > 0.02).any(axis=1).mean()
    return bad < 0.01


def kernel(x, noise, palette):
    x = np.ascontiguousarray(np.asarray(x, dtype=np.float32))
    noise = np.ascontiguousarray(np.asarray(noise, dtype=np.float32))
    palette = np.ascontiguousarray(np.asarray(palette, dtype=np.float32))
    in_maps = [
        {
            "x": x[i * B_PER_CORE : (i + 1) * B_PER_CORE],
            "noise": noise[i * B_PER_CORE : (i + 1) * B_PER_CORE],
            "palette": palette,
        }
        for i in range(N_CORES)
    ]
    out = None
    for attempt in range(3):
        nc = _get_nc()
        res = run_bass_kernel_spmd(nc, in_maps, list(range(N_CORES)))
        out = np.concatenate([res.results[i]["out"] for i in range(N_CORES)], axis=0)
        if _subsample_check(out, x, noise, palette):
            break
        # transient bad run/compile: drop the cached program and rebuild
        _NC_CACHE.clear()
    return out.astype(np.float32, copy=False)


if __name__ == "__main__":
    rng = np.random.default_rng(0)
    x = rng.random((B, C, H, W), dtype=np.float32)
    noise = rng.standard_normal((B, C, H, W), dtype=np.float32)
    palette = rng.random((K, C), dtype=np.float32)
    out = kernel(x, noise, palette)
    y = np.transpose(x + NOISE_SCALE * noise, (0, 2, 3, 1)).reshape(-1, 3)
    d = ((y[:, None, :] - palette[None, :, :]) ** 2).sum(-1)
    idx = np.argmin(d, axis=-1)
    expect = np.transpose(palette[idx].reshape(B, H, W, C), (0, 3, 1, 2))
    err = np.abs(out - expect).max()
    print("abs max err vs numpy argmin:", err)
    mism = (np.abs(out - expect) > 6e-3).any(axis=1).sum()
    print("pixels off by >6e-3:", mism, "/", B * H * W)
